# revision 29
# baseline (speedup 1.0000x reference)
"""Trainium2 8-core attention kernel for nn_Attention_8409545965959.

Reference computation (B=4, N=2048, C=1024, H=16 heads, Dh=64):
    qkv = x @ Wqkv; q,k,v per head
    att = softmax(where(mask>0, -1e7, q @ k^T / sqrt(Dh)))
    out = (att @ v) @ Wproj + bproj

Masked keys contribute exactly zero to the softmax (exp underflows to 0
in f32), so K/V are compacted host-side to the unmasked tokens of each
batch, padded per batch to a multiple of 128 (padded positions re-masked
on device via the exp bias). This is an exact reformulation that shrinks
the attention k-dimension from 2048 to ~1024-1152 per batch.

Sharding: tensor-parallel on heads (2 heads/core, column-parallel Wqkv),
then an AllToAll reshards the attention output from head-parallel to
sequence-parallel, and each core computes full output rows (row-parallel
proj over its 1024-row slice). Final gather is host-side concatenation.

On-device dataflow (per core, heads h0=2c, h1=2c+1):
  - activations kept transposed: qT/kT [128ch, n] from Wq/Wk-stationary
    matmuls vs host-transposed x^T; v in normal layout [n, 128ch].
  - S^T[k,q] per head via row-group-packed matmul pairs (K=Dh=64,
    tile_position (0,0)/(64,0)), both heads' scores in one PSUM tile
    [128, 1024].
  - softmax: exp via ScalarE activation (scale=1/sqrt(Dh), per-partition
    bias = -30000 on masked/padded k rows -> exact zeros), E^T in bf16.
  - Per q-block the emission is type-batched so same-shape matmuls run
    back-to-back on the PE (hidden LDWEIGHTS/drain): all score pairs
    (chasing the exp pipeline), then the O^T += v_h^T @ E^T accumulation
    chain (col-group packed M=64 at (0,0)/(0,64)), then all denominator
    ones-matmuls (M=1 at partitions 0/32) as one contiguous same-weight
    chain.
  - normalization: 1/D via reciprocal_approx_fast, broadcast to 128
    partitions with a K=2 bf16 matmul against a selector, O^T * (1/D) on
    VectorE -> bf16.
  - Four per-batch AllToAlls reshard O^T (shard = 256 q rows per dest);
    proj is Wproj-stationary producing out^T [1024, 1024] per core
    (+bias). Proj for group g runs as filler during batch g+2's
    attention (g=0,1) or late in batch 3 (g=2); only group 3 sits in
    the tail.

To keep the PE dense the emission order interleaves the next batch's QKV
matmuls into the attention step stream as independent filler work, a
burst of dummy warm-up matmuls covers the initial DMA wait, and a small
startup AllToAll absorbs inter-core launch stagger off the critical
path.

kernel(**inputs) accepts the full unsharded inputs and returns the full
[4, 2048, 1024] float32 output.
"""

import sys
import types

import numpy as np
import ml_dtypes

# If a caller enables BASS_TRACE without the axon NTFF profiling hook
# installed, concourse's trace path would fail importing
# antenv.axon_hooks. Provide a no-op fallback (never overrides a real
# module) so tracing degrades gracefully instead of crashing.
try:
    import antenv.axon_hooks  # noqa: F401
except ImportError:
    try:
        import antenv

        _ah = types.ModuleType("antenv.axon_hooks")
        _ah._hook = None
        _ah.set_axon_ntff_profile_hook = lambda h: setattr(_ah, "_hook", h)
        _ah.get_axon_ntff_profile_hook = lambda: _ah._hook
        sys.modules["antenv.axon_hooks"] = _ah
        antenv.axon_hooks = _ah
    except ImportError:
        pass

import concourse.bass as bass
import concourse.mybir as mybir
import concourse.tile as tile
from concourse import bacc
from concourse.bass_utils import run_bass_kernel_spmd

B = 4
N = 2048
C = 1024
H = 16
NCORES = 8
DH = C // H            # 64
HPC = H // NCORES      # 2 heads per core -> 128 channels/core
CPC = HPC * DH         # 128
ROWS = B * N           # 8192
QB = 512               # q block (one PSUM bank of f32)
KCH = 128              # k chunk (partitions)
NQB = N // QB          # 4
CC = C // 128          # 8 contraction chunks
SCALE = DH ** -0.5     # 0.125
MASK_BIAS = -30000.0

DT = mybir.dt.float32
BF = mybir.dt.bfloat16
NPBF = ml_dtypes.bfloat16

_CACHE: dict = {}
LAST_RESULTS = None


def _build(nkcs):
    """nkcs = per-batch number of 128-row k-chunks after compaction."""
    nks = [nkc * KCH for nkc in nkcs]
    koffs = [0]
    for nk in nks:
        koffs.append(koffs[-1] + nk)
    totk = koffs[-1]
    moffs = [0]
    for nkc in nkcs:
        moffs.append(moffs[-1] + nkc)
    totkc = moffs[-1]
    max_nk = max(nks)

    nc = bacc.Bacc("TRN2", target_bir_lowering=False, debug=False, num_devices=NCORES)

    xT = nc.dram_tensor("xT", [C, ROWS], BF, kind="ExternalInput")
    xTk = nc.dram_tensor("xTk", [C, totk], BF, kind="ExternalInput")
    wq = nc.dram_tensor("wq", [C, CPC], BF, kind="ExternalInput")
    wk = nc.dram_tensor("wk", [C, CPC], BF, kind="ExternalInput")
    wv = nc.dram_tensor("wv", [C, CPC], BF, kind="ExternalInput")
    wp = nc.dram_tensor("wp", [C, C], BF, kind="ExternalInput")
    bvec = nc.dram_tensor("bvec", [128, CC], DT, kind="ExternalInput")
    mb = nc.dram_tensor("mb", [128, totkc], DT, kind="ExternalInput")
    sel2 = nc.dram_tensor("sel2", [2, 128], BF, kind="ExternalInput")
    out_ext = nc.dram_tensor("out", [C, 2 * QB], DT, kind="ExternalOutput")

    # k blocks for the K^T qkv matmuls (moving dim <= 512), per batch
    kblocks = []
    for nk in nks:
        blocks = []
        pos = 0
        while pos < nk:
            w = min(QB, nk - pos)
            blocks.append((pos, w))
            pos += w
        kblocks.append(blocks)

    with tile.TileContext(nc) as tc:
        with (
            tc.tile_pool(name="consts", bufs=1) as consts,
            tc.tile_pool(name="xpool", bufs=2) as xpool,
            tc.tile_pool(name="kpool", bufs=2) as kpool,
            tc.tile_pool(name="qkpool", bufs=2) as qkpool,
            tc.tile_pool(name="vpool", bufs=2) as vpool,
            tc.tile_pool(name="epool", bufs=11) as epool,
            tc.tile_pool(name="npool", bufs=2) as npool,
            tc.tile_pool(name="opool", bufs=2) as opool,
            tc.tile_pool(name="dram", bufs=1, space="DRAM") as dram,
            tc.tile_pool(name="s_ps", bufs=2, space="PSUM") as s_ps,
            tc.tile_pool(name="o_ps", bufs=1, space="PSUM") as o_ps,
            tc.tile_pool(name="d_ps", bufs=1, space="PSUM") as d_ps,
            tc.tile_pool(name="aux_ps", bufs=2, space="PSUM") as aux_ps,
        ):
            # ---- persistent constants / weights
            wq_sb = consts.tile([128, CC, CPC], BF)
            wk_sb = consts.tile([128, CC, CPC], BF)
            wv_sb = consts.tile([128, CC, CPC], BF)
            wp_sb = consts.tile([128, CC, C], BF)
            bias_sb = consts.tile([128, CC], DT)
            mb_sb = consts.tile([128, totkc], DT)
            sel2_sb = consts.tile([2, 128], BF)
            ones_sb = consts.tile([128, 1], BF)
            warm_sb = consts.tile([128, QB], BF)
            nc.vector.memset(ones_sb[:], 1.0)
            nc.vector.memset(warm_sb[:], 0.0)
            # PE warm-up: dummy matmuls keep the PE array busy (and the
            # HAM clock-gate warm) while the first x/k DMAs land.
            warm_ps = aux_ps.tile([128, QB], DT, name="warm", tag="aux")
            for _ in range(44):
                nc.tensor.matmul(
                    warm_ps[:], warm_sb[:, 0:128], warm_sb[:], start=True, stop=True
                )
            nc.sync.dma_start(wq_sb[:], wq.rearrange("(cc p) m -> p cc m", p=128))
            nc.sync.dma_start(wk_sb[:], wk.rearrange("(cc p) m -> p cc m", p=128))
            nc.sync.dma_start(wv_sb[:], wv.rearrange("(cc p) m -> p cc m", p=128))
            nc.sync.dma_start(bias_sb[:], bvec[:])
            nc.sync.dma_start(mb_sb[:], mb[:])
            nc.sync.dma_start(sel2_sb[:], sel2[:])

            # AllToAll bounce buffers: one group per batch, shard = 256 q rows
            QS = QB // 2
            a2a_in = [
                dram.tile([NCORES, 128, QS], BF, name=f"a2a_in{i}", tag=f"a2a_in{i}")
                for i in range(B)
            ]
            a2a_out = [
                dram.tile([NCORES, 128, QS], BF, name=f"a2a_out{i}", tag=f"a2a_out{i}")
                for i in range(B)
            ]

            # startup alignment: absorb inter-core launch stagger on the
            # collective engine before real barriers sit on the critical path
            align_in = dram.tile([2, 4], BF, name="align_in", tag="align_in")
            align_out = dram.tile([2, 4], BF, name="align_out", tag="align_out")
            nc.sync.dma_start(align_in[:], sel2[0:2, 0:4])
            nc.gpsimd.collective_compute(
                "AllToAll",
                mybir.AluOpType.bypass,
                ins=[align_in.opt()],
                outs=[align_out.opt()],
                replica_groups=[list(range(NCORES))],
            )

            def emit_collective(grp):
                def emit():
                    nc.gpsimd.collective_compute(
                        "AllToAll",
                        mybir.AluOpType.bypass,
                        ins=[a2a_in[grp].opt()],
                        outs=[a2a_out[grp].opt()],
                        replica_groups=[list(range(NCORES))],
                    )

                return emit

            xb_tiles = {}
            kb_tiles = {}
            qkv_state = {}

            def emit_xb_load(b):
                # chunked loads (per cc, per column block) so the first
                # QKV matmuls can start before the whole batch arrives
                nk = nks[b]
                xb = xpool.tile([128, CC, N], BF, name=f"xb{b}", tag="xb")
                xs = xT[:, b * N:(b + 1) * N].rearrange("(cc p) n -> p cc n", p=128)
                kb = kpool.tile([128, CC, max_nk], BF, name=f"kb{b}", tag="kb")
                ks = xTk[:, koffs[b]:koffs[b] + nk].rearrange(
                    "(cc p) n -> p cc n", p=128
                )
                # first q-block of x and first k-block land first so the
                # leading QKV matmuls start as soon as possible
                for cc in range(CC):
                    nc.sync.dma_start(xb[:, cc, 0:QB], xs[:, cc, 0:QB])
                pos0, w0 = kblocks[b][0]
                for cc in range(CC):
                    nc.sync.dma_start(
                        kb[:, cc, pos0:pos0 + w0], ks[:, cc, pos0:pos0 + w0]
                    )
                for cc in range(CC):
                    for pos, w in kblocks[b][1:]:
                        nc.sync.dma_start(
                            kb[:, cc, pos:pos + w], ks[:, cc, pos:pos + w]
                        )
                for cc in range(CC):
                    for rb in range(1, NQB):
                        nc.sync.dma_start(
                            xb[:, cc, rb * QB:(rb + 1) * QB],
                            xs[:, cc, rb * QB:(rb + 1) * QB],
                        )
                xb_tiles[b] = xb
                kb_tiles[b] = kb

            def qkv_units(b):
                """Independent emission units for batch b's QKV (filler work)."""
                nk = nks[b]
                nkc = nkcs[b]
                xb = xb_tiles[b]
                kb = kb_tiles[b]
                qT = qkpool.tile([128, N], BF, name=f"qT{b}", tag="qT")
                kT = qkpool.tile([128, max_nk], BF, name=f"kT{b}", tag="kT")
                vt = vpool.tile([128, max(nkcs), CPC], BF, name=f"vt{b}", tag="vt")
                qkv_state[b] = (qT, kT, vt)
                units = []

                def q_unit(rb):
                    def emit():
                        ps = aux_ps.tile([128, QB], DT, name=f"psq{b}_{rb}", tag="aux")
                        for cc in range(CC):
                            nc.tensor.matmul(
                                ps[:],
                                wq_sb[:, cc, :],
                                xb[:, cc, rb * QB:(rb + 1) * QB],
                                start=cc == 0,
                                stop=cc == CC - 1,
                            )
                        nc.vector.tensor_copy(qT[:, rb * QB:(rb + 1) * QB], ps[:])

                    return emit

                def k_unit(pos, w):
                    def emit():
                        ps = aux_ps.tile([128, QB], DT, name=f"psk{b}_{pos}", tag="aux")
                        for cc in range(CC):
                            nc.tensor.matmul(
                                ps[:, 0:w],
                                wk_sb[:, cc, :],
                                kb[:, cc, pos:pos + w],
                                start=cc == 0,
                                stop=cc == CC - 1,
                            )
                        nc.vector.tensor_copy(kT[:, pos:pos + w], ps[:, 0:w])

                    return emit

                def v_unit(rc):
                    def emit():
                        ps = aux_ps.tile([128, QB], DT, name=f"psv{b}_{rc}", tag="aux")
                        for cc in range(CC):
                            nc.tensor.matmul(
                                ps[:, 0:CPC],
                                kb[:, cc, rc * KCH:(rc + 1) * KCH],
                                wv_sb[:, cc, :],
                                start=cc == 0,
                                stop=cc == CC - 1,
                            )
                        nc.vector.tensor_copy(vt[:, rc, :], ps[:, 0:CPC])

                    return emit

                for pos, w in kblocks[b]:
                    units.append(k_unit(pos, w))
                for rb in range(NQB):
                    units.append(q_unit(rb))
                for rc in range(nkc):
                    units.append(v_unit(rc))
                return units

            def attention_steps(b, carried=None):
                """Per q-block: score/exp steps, AV chain, batched D,
                then normalization. Returns (steps, carried_norm_b)."""
                nkc = nkcs[b]
                qT, kT, vt = qkv_state[b]
                steps = []
                norm_bs = []
                for qb in range(NQB):
                    o_acc = o_ps.tile([128, QB], DT, name=f"o{b}_{qb}", tag="o")
                    d_acc = d_ps.tile([128, QB], DT, name=f"d{b}_{qb}", tag="d")
                    e_tiles = {}

                    def se_step(qb=qb, kc=None, e_tiles=e_tiles):
                        e_tiles[kc] = epool.tile(
                            [128, 2 * QB], BF, name=f"e{b}_{qb}_{kc}", tag="e"
                        )
                        s2 = s_ps.tile(
                            [128, 2 * QB], DT, name=f"s{b}_{qb}_{kc}", tag="s"
                        )
                        nc.tensor.matmul(
                            s2[:, 0:QB],
                            kT[0:DH, kc * KCH:(kc + 1) * KCH],
                            qT[0:DH, qb * QB:(qb + 1) * QB],
                            start=True,
                            stop=True,
                            tile_position=(0, 0),
                        )
                        nc.tensor.matmul(
                            s2[:, QB:2 * QB],
                            kT[DH:2 * DH, kc * KCH:(kc + 1) * KCH],
                            qT[DH:2 * DH, qb * QB:(qb + 1) * QB],
                            start=True,
                            stop=True,
                            tile_position=(64, 0),
                        )
                        mcol = moffs[b] + kc
                        nc.scalar.activation(
                            e_tiles[kc][:],
                            s2[:],
                            mybir.ActivationFunctionType.Exp,
                            bias=mb_sb[:, mcol:mcol + 1],
                            scale=SCALE,
                        )

                    def av_step(qb=qb, kc=None, o_acc=o_acc, e_tiles=e_tiles):
                        e2 = e_tiles[kc]
                        st = kc == 0
                        sp = kc == nkc - 1
                        nc.tensor.matmul(
                            o_acc[0:DH, :],
                            vt[:, kc, 0:DH],
                            e2[:, 0:QB],
                            start=st,
                            stop=sp,
                            tile_position=(0, 0),
                        )
                        nc.tensor.matmul(
                            o_acc[DH:2 * DH, :],
                            vt[:, kc, DH:2 * DH],
                            e2[:, QB:2 * QB],
                            start=st,
                            stop=sp,
                            tile_position=(0, 64),
                        )

                    def d_batch(qb=qb, d_acc=d_acc, e_tiles=e_tiles):
                        # contiguous same-weight ones-chain: LDWEIGHTS once,
                        # back-to-back streaming
                        for kc in range(nkc):
                            e2 = e_tiles[kc]
                            st = kc == 0
                            sp = kc == nkc - 1
                            nc.tensor.matmul(
                                d_acc[0:1, :],
                                ones_sb[:],
                                e2[:, 0:QB],
                                start=st,
                                stop=sp,
                                tile_position=(0, 0),
                            )
                            nc.tensor.matmul(
                                d_acc[32:33, :],
                                ones_sb[:],
                                e2[:, QB:2 * QB],
                                start=st,
                                stop=sp,
                                tile_position=(0, 32),
                            )

                    state = {}

                    def norm_a(qb=qb, o_acc=o_acc, d_acc=d_acc, state=state):
                        # free the PSUM accumulators immediately
                        osb = opool.tile([128, QB], DT, name=f"osb{b}_{qb}", tag="osb")
                        nc.vector.tensor_copy(osb[:], o_acc[:])
                        dstage = npool.tile([33, QB], DT, name=f"dst{b}_{qb}", tag="dstage")
                        nc.vector.tensor_copy(dstage[:], d_acc[0:33, :])
                        state["osb"] = osb
                        state["dstage"] = dstage

                    def norm_b(qb=qb, state=state):
                        # deferred: the dd-DMA/reciprocal chain latency hides
                        # behind the next q-block's attention matmuls. For the
                        # batch-final q-block the staging DMAs go out on the
                        # Scalar queue (idle at batch boundaries) instead of
                        # the congested SP queue — this chain gates the
                        # inter-batch a2a.
                        osb = state["osb"]
                        dstage = state["dstage"]
                        dq = nc.scalar if qb == NQB - 1 else nc.sync
                        dd = npool.tile([2, QB], DT, name=f"dd{b}_{qb}", tag="dd")
                        dq.dma_start(dd[0:1, :], dstage[0:1, :])
                        dq.dma_start(dd[1:2, :], dstage[32:33, :])
                        dr = npool.tile([2, QB], DT, name=f"dr{b}_{qb}", tag="dr")
                        nc.vector.reciprocal_approx_fast(dr[:], dd[:])
                        drbf = npool.tile([2, QB], BF, name=f"drbf{b}_{qb}", tag="drbf")
                        nc.vector.tensor_copy(drbf[:], dr[:])
                        drb_ps = aux_ps.tile([128, QB], DT, name=f"drp{b}_{qb}", tag="aux")
                        nc.tensor.matmul(
                            drb_ps[:], sel2_sb[:], drbf[:], start=True, stop=True
                        )
                        of = opool.tile([128, QB], BF, name=f"of{b}_{qb}", tag="of")
                        nc.vector.tensor_mul(of[:], osb[:], drb_ps[:])
                        nc.sync.dma_start(
                            a2a_in[b][2 * qb, :, :], of[:, 0:QS]
                        )
                        nc.sync.dma_start(
                            a2a_in[b][2 * qb + 1, :, :], of[:, QS:QB]
                        )

                    for kc in range(nkc):
                        steps.append(lambda se=se_step, kc=kc: se(kc=kc))
                    for kc in range(nkc):
                        steps.append(lambda av=av_step, kc=kc: av(kc=kc))
                    steps.append(d_batch)
                    steps.append(norm_a)
                    norm_bs.append(norm_b)

                # norm_b(qb) lands a couple of steps into qb+1's stream; the
                # last qb's norm_b is returned so the caller can hide it in
                # the NEXT batch.
                nkc_b = nkcs[b]
                per_qb = 2 * nkc_b + 2
                woven = []
                pending_b = carried
                for i, s in enumerate(steps):
                    woven.append(s)
                    # position 5: far enough in that the dd/reciprocal chain
                    # has runtime slack before the PE stream reaches the
                    # broadcast matmul (the static schedule underestimates
                    # the staging-DMA queue delay)
                    if pending_b is not None and i % per_qb == 9:
                        woven.append(pending_b)
                        pending_b = None
                    if i % per_qb == per_qb - 1 and norm_bs:
                        pending_b = norm_bs.pop(0)
                return woven, pending_b

            def proj_load(grp):
                def load_unit():
                    ofull = qkpool.tile(
                        [128, CC, QS], BF, name=f"ofull{grp}", tag="ofull"
                    )
                    qkv_state[f"ofull{grp}"] = ofull
                    # issue on the gpsimd queue: the sequencer holds until the
                    # preceding AllToAll (same queue) has fully completed, so
                    # this cannot read a2a_out before remote shards arrive
                    nc.gpsimd.dma_start(
                        ofull[:], a2a_out[grp].rearrange("i p j -> p i j")
                    )

                return load_unit

            def proj_ocs(grp):
                units = []

                def oc_unit(oc):
                    def emit():
                        ofull = qkv_state[f"ofull{grp}"]
                        pps = aux_ps.tile([128, QB], DT, name=f"pp{grp}_{oc}", tag="aux")
                        for cc in range(CC):
                            nc.tensor.matmul(
                                pps[:, 0:QS],
                                wp_sb[:, cc, oc * 128:(oc + 1) * 128],
                                ofull[:, cc, :],
                                start=cc == 0,
                                stop=cc == CC - 1,
                            )
                        fo = npool.tile([128, QS], DT, name=f"fo{grp}_{oc}", tag="fo")
                        nc.vector.tensor_scalar_add(
                            fo[:], pps[:, 0:QS], bias_sb[:, oc:oc + 1]
                        )
                        nc.sync.dma_start(
                            out_ext[oc * 128:(oc + 1) * 128, grp * QS:(grp + 1) * QS],
                            fo[:],
                        )

                    return emit

                for oc in range(CC):
                    units.append(oc_unit(oc))
                return units

            def proj_units(grp):
                return [proj_load(grp)] + proj_ocs(grp)

            def run_interleaved(steps, fillers):
                nf = len(fillers)
                ns = len(steps)
                fi = 0
                for i, s in enumerate(steps):
                    s()
                    while fi < nf and (i + 1) * nf >= (fi + 1) * ns:
                        fillers[fi]()
                        fi += 1
                while fi < nf:
                    fillers[fi]()
                    fi += 1

            # ---- schedule: collective for batch b is emitted early in batch
            # b+1's attention; proj for group g runs as filler in batch g+2
            # (g=0,1) or late in batch 3 (g=2). Batch 0 starts attention as
            # soon as its QKV exists; batch b+1's QKV runs as filler inside
            # batch b's attention.
            emit_xb_load(0)
            # wp (2MB) is only needed for proj (batch 2 onward); queue it
            # behind batch 0's activations so they land sooner
            nc.sync.dma_start(wp_sb[:], wp.rearrange("(cc p) m -> p cc m", p=128))
            for u in qkv_units(0):
                u()
            pending = None
            carried = None
            for b in range(B):
                fillers = []
                if b < B - 1:
                    emit_xb_load(b + 1)
                if b < B - 2:
                    # batch b+1's QKV as filler inside batch b's attention
                    fillers.extend(qkv_units(b + 1))
                if b == 2:
                    fillers.extend(proj_units(0))
                if b == B - 1:
                    # batch 3's QKV is emitted up front (emission order is
                    # what dependency tracking sees — every unit must be
                    # emitted before the attention step that reads it); the
                    # scheduler still interleaves execution. This moves ~18us
                    # of PE work out of batch 2 into batch 3, whose attention
                    # is otherwise exp-bound, and leaves the tail proj groups
                    # to cover the final a2a wait.
                    for u in qkv_units(b):
                        u()
                    fillers.append(proj_load(1))
                steps, carried = attention_steps(b, carried)
                if pending is not None:
                    # position 11: after the carried norm_b (woven at 10) so the
                    # collective's input buffer is written first
                    steps.insert(11, pending)
                    pending = None
                run_interleaved(steps, fillers)
                pending = emit_collective(b)
            if carried is not None:
                carried()
            pending()
            # proj groups 1/2 are ready (their AllToAlls completed earlier) —
            # the greedy scheduler pulls them into PE idle slots of batch 3's
            # attention and the a2a(3) wait; only group 3 is gated on the
            # final collective.
            proj_load(2)()
            for g in (1, 2):
                for u in proj_ocs(g):
                    u()
            for u in proj_units(3):
                u()

    nc.compile()
    return nc


def _prep_inputs(x, Wqkv, Wproj, bproj, mask, nkcs):
    x = np.asarray(x, dtype=np.float32)
    Wqkv = np.asarray(Wqkv, dtype=np.float32)
    Wproj = np.asarray(Wproj, dtype=np.float32)
    bproj = np.asarray(bproj, dtype=np.float32)
    mask = np.asarray(mask)
    nks = [nkc * KCH for nkc in nkcs]
    koffs = np.concatenate([[0], np.cumsum(nks)]).astype(int)
    moffs = np.concatenate([[0], np.cumsum(nkcs)]).astype(int)
    totk = int(koffs[-1])
    totkc = int(moffs[-1])

    x2 = x.reshape(ROWS, C)
    xT = np.ascontiguousarray(x2.T).astype(NPBF)
    # compacted K/V tokens: unmasked columns per batch, zero-padded to nk_b
    xTk = np.zeros((C, totk), dtype=NPBF)
    mbias = np.full((totk,), np.float32(MASK_BIAS), dtype=np.float32)
    for b in range(B):
        idx = np.nonzero(mask[b] == 0)[0]
        cnt = len(idx)
        xTk[:, koffs[b]: koffs[b] + cnt] = xT[:, b * N + idx]
        mbias[koffs[b]: koffs[b] + cnt] = 0.0
    mb_arr = np.zeros((128, totkc), dtype=np.float32)
    for b in range(B):
        blk = mbias[koffs[b]:koffs[b + 1]].reshape(nkcs[b], 128).T
        mb_arr[:, moffs[b]:moffs[b + 1]] = blk

    wp_bf = Wproj.astype(NPBF)
    bias_r = np.ascontiguousarray(bproj.reshape(CC, 128).T).astype(np.float32)
    sel2 = np.zeros((2, 128), np.float32)
    sel2[0, 0:64] = 1.0
    sel2[1, 64:128] = 1.0
    sel2 = sel2.astype(NPBF)

    in_maps = []
    for c in range(NCORES):
        cols = slice(c * CPC, (c + 1) * CPC)
        in_maps.append(
            dict(
                xT=xT,
                xTk=xTk,
                wq=np.ascontiguousarray(Wqkv[:, cols]).astype(NPBF),
                wk=np.ascontiguousarray(Wqkv[:, C:][:, cols]).astype(NPBF),
                wv=np.ascontiguousarray(Wqkv[:, 2 * C:][:, cols]).astype(NPBF),
                wp=wp_bf,
                bvec=bias_r,
                mb=mb_arr,
                sel2=sel2,
            )
        )
    return in_maps


def kernel(x, Wqkv, Wproj, bproj, mask):
    global LAST_RESULTS
    mask = np.asarray(mask)
    cnts = (mask == 0).sum(axis=1)
    nkcs = tuple(max(1, int(-(-c // KCH))) for c in cnts)
    if nkcs not in _CACHE:
        _CACHE[nkcs] = _build(nkcs)
    nc = _CACHE[nkcs]
    in_maps = _prep_inputs(x, Wqkv, Wproj, bproj, mask, nkcs)
    res = run_bass_kernel_spmd(nc, in_maps, list(range(NCORES)))
    LAST_RESULTS = res
    out = np.empty((ROWS, C), dtype=np.float32)
    QS = QB // 2
    for c in range(NCORES):
        oT = res.results[c]["out"]  # [1024 oc, 4*256 q] = final^T slice
        for b in range(B):
            rows = slice(b * N + c * QS, b * N + (c + 1) * QS)
            out[rows, :] = oT[:, b * QS:(b + 1) * QS].T
    return out.reshape(B, N, C)


# revision 31
# speedup vs baseline: 1.0041x; 1.0041x over previous
"""Trainium2 8-core attention kernel for nn_Attention_8409545965959.

Reference computation (B=4, N=2048, C=1024, H=16 heads, Dh=64):
    qkv = x @ Wqkv; q,k,v per head
    att = softmax(where(mask>0, -1e7, q @ k^T / sqrt(Dh)))
    out = (att @ v) @ Wproj + bproj

Masked keys contribute exactly zero to the softmax (exp underflows to 0
in f32), so K/V are compacted host-side to the unmasked tokens of each
batch, padded per batch to a multiple of 128 (padded positions re-masked
on device via the exp bias). This is an exact reformulation that shrinks
the attention k-dimension from 2048 to ~1024-1152 per batch.

Sharding: tensor-parallel on heads (2 heads/core, column-parallel Wqkv),
then an AllToAll reshards the attention output from head-parallel to
sequence-parallel, and each core computes full output rows (row-parallel
proj over its 1024-row slice). Final gather is host-side concatenation.

On-device dataflow (per core, heads h0=2c, h1=2c+1):
  - activations kept transposed: qT/kT [128ch, n] from Wq/Wk-stationary
    matmuls vs host-transposed x^T; v in normal layout [n, 128ch].
  - S^T[k,q] per head via row-group-packed matmul pairs (K=Dh=64,
    tile_position (0,0)/(64,0)), both heads' scores in one PSUM tile
    [128, 1024].
  - softmax: exp via ScalarE activation (scale=1/sqrt(Dh), per-partition
    bias = -30000 on masked/padded k rows -> exact zeros), E^T in bf16.
  - Per q-block the emission is type-batched so same-shape matmuls run
    back-to-back on the PE (hidden LDWEIGHTS/drain): all score pairs
    (chasing the exp pipeline), then the O^T += v_h^T @ E^T accumulation
    chain (col-group packed M=64 at (0,0)/(0,64)), then all denominator
    ones-matmuls (M=1 at partitions 0/32) as one contiguous same-weight
    chain.
  - normalization: 1/D via reciprocal_approx_fast, broadcast to 128
    partitions with a K=2 bf16 matmul against a selector, O^T * (1/D) on
    VectorE -> bf16.
  - Four per-batch AllToAlls reshard O^T (shard = 256 q rows per dest);
    proj is Wproj-stationary producing out^T [1024, 1024] per core
    (+bias). Proj for group g runs as filler during batch g+2's
    attention (g=0,1) or late in batch 3 (g=2); only group 3 sits in
    the tail.

To keep the PE dense the emission order interleaves the next batch's QKV
matmuls into the attention step stream as independent filler work, a
burst of dummy warm-up matmuls covers the initial DMA wait, and a small
startup AllToAll absorbs inter-core launch stagger off the critical
path.

kernel(**inputs) accepts the full unsharded inputs and returns the full
[4, 2048, 1024] float32 output.
"""

import sys
import types

import numpy as np
import ml_dtypes

# If a caller enables BASS_TRACE without the axon NTFF profiling hook
# installed, concourse's trace path would fail importing
# antenv.axon_hooks. Provide a no-op fallback (never overrides a real
# module) so tracing degrades gracefully instead of crashing.
try:
    import antenv.axon_hooks  # noqa: F401
except ImportError:
    try:
        import antenv

        _ah = types.ModuleType("antenv.axon_hooks")
        _ah._hook = None
        _ah.set_axon_ntff_profile_hook = lambda h: setattr(_ah, "_hook", h)
        _ah.get_axon_ntff_profile_hook = lambda: _ah._hook
        sys.modules["antenv.axon_hooks"] = _ah
        antenv.axon_hooks = _ah
    except ImportError:
        pass

import concourse.bass as bass
import concourse.mybir as mybir
import concourse.tile as tile
from concourse import bacc
from concourse.bass_utils import run_bass_kernel_spmd

B = 4
N = 2048
C = 1024
H = 16
NCORES = 8
DH = C // H            # 64
HPC = H // NCORES      # 2 heads per core -> 128 channels/core
CPC = HPC * DH         # 128
ROWS = B * N           # 8192
QB = 512               # q block (one PSUM bank of f32)
KCH = 128              # k chunk (partitions)
NQB = N // QB          # 4
CC = C // 128          # 8 contraction chunks
SCALE = DH ** -0.5     # 0.125
MASK_BIAS = -30000.0

DT = mybir.dt.float32
BF = mybir.dt.bfloat16
NPBF = ml_dtypes.bfloat16

_CACHE: dict = {}
LAST_RESULTS = None


def _build(nkcs):
    """nkcs = per-batch number of 128-row k-chunks after compaction."""
    nks = [nkc * KCH for nkc in nkcs]
    koffs = [0]
    for nk in nks:
        koffs.append(koffs[-1] + nk)
    totk = koffs[-1]
    moffs = [0]
    for nkc in nkcs:
        moffs.append(moffs[-1] + nkc)
    totkc = moffs[-1]
    max_nk = max(nks)

    nc = bacc.Bacc("TRN2", target_bir_lowering=False, debug=False, num_devices=NCORES)

    xT = nc.dram_tensor("xT", [C, ROWS], BF, kind="ExternalInput")
    xTk = nc.dram_tensor("xTk", [C, totk], BF, kind="ExternalInput")
    wq = nc.dram_tensor("wq", [C, CPC], BF, kind="ExternalInput")
    wk = nc.dram_tensor("wk", [C, CPC], BF, kind="ExternalInput")
    wv = nc.dram_tensor("wv", [C, CPC], BF, kind="ExternalInput")
    wp = nc.dram_tensor("wp", [C, C], BF, kind="ExternalInput")
    bvec = nc.dram_tensor("bvec", [128, CC], DT, kind="ExternalInput")
    mb = nc.dram_tensor("mb", [128, totkc], DT, kind="ExternalInput")
    sel2 = nc.dram_tensor("sel2", [2, 128], BF, kind="ExternalInput")
    out_ext = nc.dram_tensor("out", [C, 2 * QB], DT, kind="ExternalOutput")

    # k blocks for the K^T qkv matmuls (moving dim <= 512), per batch
    kblocks = []
    for nk in nks:
        blocks = []
        pos = 0
        while pos < nk:
            w = min(QB, nk - pos)
            blocks.append((pos, w))
            pos += w
        kblocks.append(blocks)

    with tile.TileContext(nc) as tc:
        with (
            tc.tile_pool(name="consts", bufs=1) as consts,
            tc.tile_pool(name="xpool", bufs=2) as xpool,
            tc.tile_pool(name="kpool", bufs=2) as kpool,
            tc.tile_pool(name="qkpool", bufs=2) as qkpool,
            tc.tile_pool(name="vpool", bufs=2) as vpool,
            tc.tile_pool(name="epool", bufs=11) as epool,
            tc.tile_pool(name="npool", bufs=2) as npool,
            tc.tile_pool(name="opool", bufs=2) as opool,
            tc.tile_pool(name="dram", bufs=1, space="DRAM") as dram,
            tc.tile_pool(name="s_ps", bufs=2, space="PSUM") as s_ps,
            tc.tile_pool(name="o_ps", bufs=1, space="PSUM") as o_ps,
            tc.tile_pool(name="d_ps", bufs=1, space="PSUM") as d_ps,
            tc.tile_pool(name="aux_ps", bufs=2, space="PSUM") as aux_ps,
        ):
            # ---- persistent constants / weights
            wq_sb = consts.tile([128, CC, CPC], BF)
            wk_sb = consts.tile([128, CC, CPC], BF)
            wv_sb = consts.tile([128, CC, CPC], BF)
            wp_sb = consts.tile([128, CC, C], BF)
            bias_sb = consts.tile([128, CC], DT)
            mb_sb = consts.tile([128, totkc], DT)
            sel2_sb = consts.tile([2, 128], BF)
            ones_sb = consts.tile([128, 1], BF)
            warm_sb = consts.tile([128, QB], BF)
            nc.vector.memset(ones_sb[:], 1.0)
            nc.vector.memset(warm_sb[:], 0.0)
            # PE warm-up: dummy matmuls keep the PE array busy (and the
            # HAM clock-gate warm) while the first x/k DMAs land.
            warm_ps = aux_ps.tile([128, QB], DT, name="warm", tag="aux")
            for _ in range(36):
                nc.tensor.matmul(
                    warm_ps[:], warm_sb[:, 0:128], warm_sb[:], start=True, stop=True
                )
            nc.sync.dma_start(wq_sb[:], wq.rearrange("(cc p) m -> p cc m", p=128))
            nc.sync.dma_start(wk_sb[:], wk.rearrange("(cc p) m -> p cc m", p=128))
            nc.sync.dma_start(wv_sb[:], wv.rearrange("(cc p) m -> p cc m", p=128))
            nc.sync.dma_start(bias_sb[:], bvec[:])
            nc.sync.dma_start(mb_sb[:], mb[:])
            nc.sync.dma_start(sel2_sb[:], sel2[:])

            # AllToAll bounce buffers: one group per batch, shard = 256 q rows
            QS = QB // 2
            a2a_in = [
                dram.tile([NCORES, 128, QS], BF, name=f"a2a_in{i}", tag=f"a2a_in{i}")
                for i in range(B)
            ]
            a2a_out = [
                dram.tile([NCORES, 128, QS], BF, name=f"a2a_out{i}", tag=f"a2a_out{i}")
                for i in range(B)
            ]

            # startup alignment: absorb inter-core launch stagger on the
            # collective engine before real barriers sit on the critical path
            align_in = dram.tile([2, 4], BF, name="align_in", tag="align_in")
            align_out = dram.tile([2, 4], BF, name="align_out", tag="align_out")
            nc.sync.dma_start(align_in[:], sel2[0:2, 0:4])
            nc.gpsimd.collective_compute(
                "AllToAll",
                mybir.AluOpType.bypass,
                ins=[align_in.opt()],
                outs=[align_out.opt()],
                replica_groups=[list(range(NCORES))],
            )

            def emit_collective(grp):
                def emit():
                    nc.gpsimd.collective_compute(
                        "AllToAll",
                        mybir.AluOpType.bypass,
                        ins=[a2a_in[grp].opt()],
                        outs=[a2a_out[grp].opt()],
                        replica_groups=[list(range(NCORES))],
                    )

                return emit

            xb_tiles = {}
            kb_tiles = {}
            qkv_state = {}

            def emit_xb_load(b):
                # chunked loads (per cc, per column block) so the first
                # QKV matmuls can start before the whole batch arrives
                nk = nks[b]
                xb = xpool.tile([128, CC, N], BF, name=f"xb{b}", tag="xb")
                xs = xT[:, b * N:(b + 1) * N].rearrange("(cc p) n -> p cc n", p=128)
                kb = kpool.tile([128, CC, max_nk], BF, name=f"kb{b}", tag="kb")
                ks = xTk[:, koffs[b]:koffs[b] + nk].rearrange(
                    "(cc p) n -> p cc n", p=128
                )
                # first q-block of x and first k-block land first so the
                # leading QKV matmuls start as soon as possible
                for cc in range(CC):
                    nc.sync.dma_start(xb[:, cc, 0:QB], xs[:, cc, 0:QB])
                pos0, w0 = kblocks[b][0]
                for cc in range(CC):
                    nc.sync.dma_start(
                        kb[:, cc, pos0:pos0 + w0], ks[:, cc, pos0:pos0 + w0]
                    )
                for cc in range(CC):
                    for pos, w in kblocks[b][1:]:
                        nc.sync.dma_start(
                            kb[:, cc, pos:pos + w], ks[:, cc, pos:pos + w]
                        )
                for cc in range(CC):
                    for rb in range(1, NQB):
                        nc.sync.dma_start(
                            xb[:, cc, rb * QB:(rb + 1) * QB],
                            xs[:, cc, rb * QB:(rb + 1) * QB],
                        )
                xb_tiles[b] = xb
                kb_tiles[b] = kb

            def qkv_units(b):
                """Independent emission units for batch b's QKV (filler work)."""
                nk = nks[b]
                nkc = nkcs[b]
                xb = xb_tiles[b]
                kb = kb_tiles[b]
                qT = qkpool.tile([128, N], BF, name=f"qT{b}", tag="qT")
                kT = qkpool.tile([128, max_nk], BF, name=f"kT{b}", tag="kT")
                vt = vpool.tile([128, max(nkcs), CPC], BF, name=f"vt{b}", tag="vt")
                qkv_state[b] = (qT, kT, vt)
                units = []

                def q_unit(rb):
                    def emit():
                        ps = aux_ps.tile([128, QB], DT, name=f"psq{b}_{rb}", tag="aux")
                        for cc in range(CC):
                            nc.tensor.matmul(
                                ps[:],
                                wq_sb[:, cc, :],
                                xb[:, cc, rb * QB:(rb + 1) * QB],
                                start=cc == 0,
                                stop=cc == CC - 1,
                            )
                        nc.vector.tensor_copy(qT[:, rb * QB:(rb + 1) * QB], ps[:])

                    return emit

                def k_unit(pos, w):
                    def emit():
                        ps = aux_ps.tile([128, QB], DT, name=f"psk{b}_{pos}", tag="aux")
                        for cc in range(CC):
                            nc.tensor.matmul(
                                ps[:, 0:w],
                                wk_sb[:, cc, :],
                                kb[:, cc, pos:pos + w],
                                start=cc == 0,
                                stop=cc == CC - 1,
                            )
                        nc.vector.tensor_copy(kT[:, pos:pos + w], ps[:, 0:w])

                    return emit

                def v_unit(rc):
                    def emit():
                        ps = aux_ps.tile([128, QB], DT, name=f"psv{b}_{rc}", tag="aux")
                        for cc in range(CC):
                            nc.tensor.matmul(
                                ps[:, 0:CPC],
                                kb[:, cc, rc * KCH:(rc + 1) * KCH],
                                wv_sb[:, cc, :],
                                start=cc == 0,
                                stop=cc == CC - 1,
                            )
                        nc.vector.tensor_copy(vt[:, rc, :], ps[:, 0:CPC])

                    return emit

                for pos, w in kblocks[b]:
                    units.append(k_unit(pos, w))
                for rb in range(NQB):
                    units.append(q_unit(rb))
                for rc in range(nkc):
                    units.append(v_unit(rc))
                return units

            def attention_steps(b, carried=None):
                """Per q-block: score/exp steps, AV chain, batched D,
                then normalization. Returns (steps, carried_norm_b)."""
                nkc = nkcs[b]
                qT, kT, vt = qkv_state[b]
                steps = []
                norm_bs = []
                for qb in range(NQB):
                    o_acc = o_ps.tile([128, QB], DT, name=f"o{b}_{qb}", tag="o")
                    d_acc = d_ps.tile([128, QB], DT, name=f"d{b}_{qb}", tag="d")
                    e_tiles = {}

                    def se_step(qb=qb, kc=None, e_tiles=e_tiles):
                        e_tiles[kc] = epool.tile(
                            [128, 2 * QB], BF, name=f"e{b}_{qb}_{kc}", tag="e"
                        )
                        s2 = s_ps.tile(
                            [128, 2 * QB], DT, name=f"s{b}_{qb}_{kc}", tag="s"
                        )
                        nc.tensor.matmul(
                            s2[:, 0:QB],
                            kT[0:DH, kc * KCH:(kc + 1) * KCH],
                            qT[0:DH, qb * QB:(qb + 1) * QB],
                            start=True,
                            stop=True,
                            tile_position=(0, 0),
                        )
                        nc.tensor.matmul(
                            s2[:, QB:2 * QB],
                            kT[DH:2 * DH, kc * KCH:(kc + 1) * KCH],
                            qT[DH:2 * DH, qb * QB:(qb + 1) * QB],
                            start=True,
                            stop=True,
                            tile_position=(64, 0),
                        )
                        mcol = moffs[b] + kc
                        nc.scalar.activation(
                            e_tiles[kc][:],
                            s2[:],
                            mybir.ActivationFunctionType.Exp,
                            bias=mb_sb[:, mcol:mcol + 1],
                            scale=SCALE,
                        )

                    def av_step(qb=qb, kc=None, o_acc=o_acc, e_tiles=e_tiles):
                        e2 = e_tiles[kc]
                        st = kc == 0
                        sp = kc == nkc - 1
                        nc.tensor.matmul(
                            o_acc[0:DH, :],
                            vt[:, kc, 0:DH],
                            e2[:, 0:QB],
                            start=st,
                            stop=sp,
                            tile_position=(0, 0),
                        )
                        nc.tensor.matmul(
                            o_acc[DH:2 * DH, :],
                            vt[:, kc, DH:2 * DH],
                            e2[:, QB:2 * QB],
                            start=st,
                            stop=sp,
                            tile_position=(0, 64),
                        )

                    def d_batch(qb=qb, d_acc=d_acc, e_tiles=e_tiles):
                        # contiguous same-weight ones-chain: LDWEIGHTS once,
                        # back-to-back streaming
                        for kc in range(nkc):
                            e2 = e_tiles[kc]
                            st = kc == 0
                            sp = kc == nkc - 1
                            nc.tensor.matmul(
                                d_acc[0:1, :],
                                ones_sb[:],
                                e2[:, 0:QB],
                                start=st,
                                stop=sp,
                                tile_position=(0, 0),
                            )
                            nc.tensor.matmul(
                                d_acc[32:33, :],
                                ones_sb[:],
                                e2[:, QB:2 * QB],
                                start=st,
                                stop=sp,
                                tile_position=(0, 32),
                            )

                    state = {}

                    def norm_a(qb=qb, o_acc=o_acc, d_acc=d_acc, state=state):
                        # free the PSUM accumulators immediately
                        osb = opool.tile([128, QB], DT, name=f"osb{b}_{qb}", tag="osb")
                        nc.vector.tensor_copy(osb[:], o_acc[:])
                        dstage = npool.tile([33, QB], DT, name=f"dst{b}_{qb}", tag="dstage")
                        nc.vector.tensor_copy(dstage[:], d_acc[0:33, :])
                        state["osb"] = osb
                        state["dstage"] = dstage

                    def norm_b(qb=qb, state=state):
                        # deferred: the dd-DMA/reciprocal chain latency hides
                        # behind the next q-block's attention matmuls. For the
                        # batch-final q-block the staging DMAs go out on the
                        # Scalar queue (idle at batch boundaries) instead of
                        # the congested SP queue — this chain gates the
                        # inter-batch a2a.
                        osb = state["osb"]
                        dstage = state["dstage"]
                        dq = nc.scalar if qb == NQB - 1 else nc.sync
                        dd = npool.tile([2, QB], DT, name=f"dd{b}_{qb}", tag="dd")
                        dq.dma_start(dd[0:1, :], dstage[0:1, :])
                        dq.dma_start(dd[1:2, :], dstage[32:33, :])
                        dr = npool.tile([2, QB], DT, name=f"dr{b}_{qb}", tag="dr")
                        nc.vector.reciprocal_approx_fast(dr[:], dd[:])
                        drbf = npool.tile([2, QB], BF, name=f"drbf{b}_{qb}", tag="drbf")
                        nc.vector.tensor_copy(drbf[:], dr[:])
                        drb_ps = aux_ps.tile([128, QB], DT, name=f"drp{b}_{qb}", tag="aux")
                        nc.tensor.matmul(
                            drb_ps[:], sel2_sb[:], drbf[:], start=True, stop=True
                        )
                        of = opool.tile([128, QB], BF, name=f"of{b}_{qb}", tag="of")
                        nc.vector.tensor_mul(of[:], osb[:], drb_ps[:])
                        nc.sync.dma_start(
                            a2a_in[b][2 * qb, :, :], of[:, 0:QS]
                        )
                        nc.sync.dma_start(
                            a2a_in[b][2 * qb + 1, :, :], of[:, QS:QB]
                        )

                    for kc in range(nkc):
                        steps.append(lambda se=se_step, kc=kc: se(kc=kc))
                    for kc in range(nkc):
                        steps.append(lambda av=av_step, kc=kc: av(kc=kc))
                    steps.append(d_batch)
                    steps.append(norm_a)
                    norm_bs.append(norm_b)

                # norm_b(qb) lands a couple of steps into qb+1's stream; the
                # last qb's norm_b is returned so the caller can hide it in
                # the NEXT batch.
                nkc_b = nkcs[b]
                per_qb = 2 * nkc_b + 2
                woven = []
                pending_b = carried
                for i, s in enumerate(steps):
                    woven.append(s)
                    # position 5: far enough in that the dd/reciprocal chain
                    # has runtime slack before the PE stream reaches the
                    # broadcast matmul (the static schedule underestimates
                    # the staging-DMA queue delay)
                    if pending_b is not None and i % per_qb == 5:
                        woven.append(pending_b)
                        pending_b = None
                    if i % per_qb == per_qb - 1 and norm_bs:
                        pending_b = norm_bs.pop(0)
                return woven, pending_b

            def proj_load(grp):
                def load_unit():
                    ofull = qkpool.tile(
                        [128, CC, QS], BF, name=f"ofull{grp}", tag="ofull"
                    )
                    qkv_state[f"ofull{grp}"] = ofull
                    # issue on the gpsimd queue: the sequencer holds until the
                    # preceding AllToAll (same queue) has fully completed, so
                    # this cannot read a2a_out before remote shards arrive
                    nc.gpsimd.dma_start(
                        ofull[:], a2a_out[grp].rearrange("i p j -> p i j")
                    )

                return load_unit

            def proj_ocs(grp):
                units = []

                def oc_unit(oc):
                    def emit():
                        ofull = qkv_state[f"ofull{grp}"]
                        pps = aux_ps.tile([128, QB], DT, name=f"pp{grp}_{oc}", tag="aux")
                        for cc in range(CC):
                            nc.tensor.matmul(
                                pps[:, 0:QS],
                                wp_sb[:, cc, oc * 128:(oc + 1) * 128],
                                ofull[:, cc, :],
                                start=cc == 0,
                                stop=cc == CC - 1,
                            )
                        fo = npool.tile([128, QS], DT, name=f"fo{grp}_{oc}", tag="fo")
                        nc.vector.tensor_scalar_add(
                            fo[:], pps[:, 0:QS], bias_sb[:, oc:oc + 1]
                        )
                        nc.sync.dma_start(
                            out_ext[oc * 128:(oc + 1) * 128, grp * QS:(grp + 1) * QS],
                            fo[:],
                        )

                    return emit

                for oc in range(CC):
                    units.append(oc_unit(oc))
                return units

            def proj_units(grp):
                return [proj_load(grp)] + proj_ocs(grp)

            def run_interleaved(steps, fillers):
                nf = len(fillers)
                ns = len(steps)
                fi = 0
                for i, s in enumerate(steps):
                    s()
                    while fi < nf and (i + 1) * nf >= (fi + 1) * ns:
                        fillers[fi]()
                        fi += 1
                while fi < nf:
                    fillers[fi]()
                    fi += 1

            # ---- schedule: collective for batch b is emitted early in batch
            # b+1's attention; proj for group g runs as filler in batch g+2
            # (g=0,1) or late in batch 3 (g=2). Batch 0 starts attention as
            # soon as its QKV exists; batch b+1's QKV runs as filler inside
            # batch b's attention.
            emit_xb_load(0)
            # wp (2MB) is only needed for proj (batch 2 onward); queue it
            # behind batch 0's activations so they land sooner
            nc.sync.dma_start(wp_sb[:], wp.rearrange("(cc p) m -> p cc m", p=128))
            for u in qkv_units(0):
                u()
            pending = None
            carried = None
            for b in range(B):
                fillers = []
                if b < B - 1:
                    emit_xb_load(b + 1)
                if b < B - 2:
                    # batch b+1's QKV as filler inside batch b's attention
                    fillers.extend(qkv_units(b + 1))
                if b == 2:
                    fillers.extend(proj_units(0))
                if b == B - 1:
                    # batch 3's QKV is emitted up front (emission order is
                    # what dependency tracking sees — every unit must be
                    # emitted before the attention step that reads it); the
                    # scheduler still interleaves execution. This moves ~18us
                    # of PE work out of batch 2 into batch 3, whose attention
                    # is otherwise exp-bound, and leaves the tail proj groups
                    # to cover the final a2a wait.
                    for u in qkv_units(b):
                        u()
                    fillers.append(proj_load(1))
                steps, carried = attention_steps(b, carried)
                if pending is not None:
                    # position 7: after the carried norm_b (woven at 6) so the
                    # collective's input buffer is written first
                    steps.insert(7, pending)
                    pending = None
                run_interleaved(steps, fillers)
                pending = emit_collective(b)
            if carried is not None:
                carried()
            # load(2) must precede the final collective on the in-order
            # gpsimd queue: it only needs a2a(2) (completed mid-batch-3);
            # queued after a2a(3) it would block proj(2)'s matmuls from
            # covering the final collective's wait window
            proj_load(2)()
            pending()
            # proj groups 1/2 are ready — the greedy scheduler pulls them
            # into PE idle slots of batch 3's attention and the a2a(3) wait;
            # only group 3 is gated on the final collective.
            for g in (1, 2):
                for u in proj_ocs(g):
                    u()
            for u in proj_units(3):
                u()

    nc.compile()
    return nc


def _prep_inputs(x, Wqkv, Wproj, bproj, mask, nkcs):
    x = np.asarray(x, dtype=np.float32)
    Wqkv = np.asarray(Wqkv, dtype=np.float32)
    Wproj = np.asarray(Wproj, dtype=np.float32)
    bproj = np.asarray(bproj, dtype=np.float32)
    mask = np.asarray(mask)
    nks = [nkc * KCH for nkc in nkcs]
    koffs = np.concatenate([[0], np.cumsum(nks)]).astype(int)
    moffs = np.concatenate([[0], np.cumsum(nkcs)]).astype(int)
    totk = int(koffs[-1])
    totkc = int(moffs[-1])

    x2 = x.reshape(ROWS, C)
    xT = np.ascontiguousarray(x2.T).astype(NPBF)
    # compacted K/V tokens: unmasked columns per batch, zero-padded to nk_b
    xTk = np.zeros((C, totk), dtype=NPBF)
    mbias = np.full((totk,), np.float32(MASK_BIAS), dtype=np.float32)
    for b in range(B):
        idx = np.nonzero(mask[b] == 0)[0]
        cnt = len(idx)
        xTk[:, koffs[b]: koffs[b] + cnt] = xT[:, b * N + idx]
        mbias[koffs[b]: koffs[b] + cnt] = 0.0
    mb_arr = np.zeros((128, totkc), dtype=np.float32)
    for b in range(B):
        blk = mbias[koffs[b]:koffs[b + 1]].reshape(nkcs[b], 128).T
        mb_arr[:, moffs[b]:moffs[b + 1]] = blk

    wp_bf = Wproj.astype(NPBF)
    bias_r = np.ascontiguousarray(bproj.reshape(CC, 128).T).astype(np.float32)
    sel2 = np.zeros((2, 128), np.float32)
    sel2[0, 0:64] = 1.0
    sel2[1, 64:128] = 1.0
    sel2 = sel2.astype(NPBF)

    in_maps = []
    for c in range(NCORES):
        cols = slice(c * CPC, (c + 1) * CPC)
        in_maps.append(
            dict(
                xT=xT,
                xTk=xTk,
                wq=np.ascontiguousarray(Wqkv[:, cols]).astype(NPBF),
                wk=np.ascontiguousarray(Wqkv[:, C:][:, cols]).astype(NPBF),
                wv=np.ascontiguousarray(Wqkv[:, 2 * C:][:, cols]).astype(NPBF),
                wp=wp_bf,
                bvec=bias_r,
                mb=mb_arr,
                sel2=sel2,
            )
        )
    return in_maps


def kernel(x, Wqkv, Wproj, bproj, mask):
    global LAST_RESULTS
    mask = np.asarray(mask)
    cnts = (mask == 0).sum(axis=1)
    nkcs = tuple(max(1, int(-(-c // KCH))) for c in cnts)
    if nkcs not in _CACHE:
        _CACHE[nkcs] = _build(nkcs)
    nc = _CACHE[nkcs]
    in_maps = _prep_inputs(x, Wqkv, Wproj, bproj, mask, nkcs)
    res = run_bass_kernel_spmd(nc, in_maps, list(range(NCORES)))
    LAST_RESULTS = res
    out = np.empty((ROWS, C), dtype=np.float32)
    QS = QB // 2
    for c in range(NCORES):
        oT = res.results[c]["out"]  # [1024 oc, 4*256 q] = final^T slice
        for b in range(B):
            rows = slice(b * N + c * QS, b * N + (c + 1) * QS)
            out[rows, :] = oT[:, b * QS:(b + 1) * QS].T
    return out.reshape(B, N, C)
